# revision 14
# baseline (speedup 1.0000x reference)
"""Trainium2 Bass kernel for the heterogeneous GNN (GAT + SAGE, 2 layers).

Strategy: destination-node sharding across 8 cores (papers 12500/core,
authors 6250/core). Each layer:
  1. Per-core dense pass over the LOCAL node shard computes projected
     gather-tables with one fused matmul per tile (rhs = [Wsrc | u | Wl]):
         F_gat rows (768B): [h @ Wsrc (128) | al_d (4, dst-side table) |
                             al_s (4) | pad]
         F_ps rows (512B):  [h @ Wl]
     al_d also lands in a 256B-padded local DRAM table (gathered per edge
     by destination index later).
  2. AllGather the F tables (all cores get full copies).
  3. Edge phase: edges sorted by destination, cut into 128-edge chunks per
     128-destination tile, sub-grouped by source-table segment (int16 index
     reach = 32000 rows). Gathers are big batched GPSIMD dma_gather ops (one
     per tile-group x segment, round-robin over 4 SWDGE queues). Per chunk:
     selection matrix S[e,j]=(dst_rel[e]==j) on the vector engine, softmax
     numerators exp(leakyrelu(al_s+al_d)) (max-shift dropped - softmax is
     shift invariant), messages scaled per head, then one matmul
     S.T @ [msg|ex] accumulating messages + softmax denominators in PSUM.
  4. Softmax division after aggregation, SAGE mean via host reciprocal
     counts, + h_dst @ Wr, LayerNorm, ReLU, residual - all shard-local.
Weights replicated; zero biases / unit LN gains elide ops at build time.
"""
import sys

if "/opt/trn_rl_repo" not in sys.path:
    sys.path.insert(0, "/opt/trn_rl_repo")

import numpy as np

NCORES = 8
NA, NP_ = 50000, 100000
H, HEADS, CH = 128, 4, 32
IN_A, IN_P = 128, 256
LN_EPS = 1e-5
L = 2
P = 128
PSH, ASH = NP_ // NCORES, NA // NCORES
PT, AT = (PSH + P - 1) // P, (ASH + P - 1) // P
SEGR = 32000     # gather-table rows per segment (int16 index reach)
EG = 192         # f32 per F_gat row (768B)
ES = 128         # f32 per F_ps row (512B)
EA = 64          # f32 per al_d row (256B)
AS0 = 132        # al_s offset inside F_gat rows
TG = 2           # dst tiles per gather group


def _wrap_idx(idx):
    """int16 idx list -> [128, n/16] (16-partition wrap, replicated 8x)."""
    n = idx.shape[0]
    assert n % 16 == 0
    w = idx.reshape(n // 16, 16).T.astype(np.int16)
    return np.tile(w, (8, 1))


def _prep_edges(src, dst, shard, n_tiles, n_src, need_ald):
    """Group edges by (dst tile, src segment), pad each (tile,seg) to a
    cross-core-uniform 128-multiple, lay out gather idx streams per
    (tile-group, seg) instruction (staging order: s-major, t-minor within
    the group) plus per-column dst_rel and optional al_d idx streams."""
    nseg = (n_src + SEGR - 1) // SEGR
    ngrp = (n_tiles + TG - 1) // TG
    src = np.asarray(src).astype(np.int64)
    dst = np.asarray(dst).astype(np.int64)
    per_core = []
    cnts = np.zeros((NCORES, n_tiles, nseg), np.int64)
    for r in range(NCORES):
        lo = r * shard
        sel = (dst >= lo) & (dst < lo + shard)
        s, d = src[sel], dst[sel] - lo
        seg = s // SEGR
        t = d >> 7
        o = np.lexsort((d, seg, t))
        s, d = s[o], d[o]
        np.add.at(cnts[r], (t[o], seg[o]), 1)
        per_core.append((s, d))
    K = ((cnts + P - 1) // P).max(axis=0)            # [n_tiles, nseg]
    # global column enumeration: for g: for s: for t in g
    colbase = np.zeros((n_tiles, nseg), np.int64)
    finst = []                                       # (g, s) -> (col0, ncols)
    tot = 0
    for g_ in range(ngrp):
        tl = list(range(g_ * TG, min((g_ + 1) * TG, n_tiles)))
        for s_ in range(nseg):
            c0 = tot
            for t_ in tl:
                colbase[t_, s_] = tot
                tot += K[t_, s_]
            finst.append((c0, tot - c0))
    ncols = tot
    tile_cols = [np.concatenate([np.arange(colbase[t_, s_],
                                           colbase[t_, s_] + K[t_, s_])
                                 for s_ in range(nseg)])
                 for t_ in range(n_tiles)]
    cores = []
    for r in range(NCORES):
        s, d = per_core[r]
        srcl = np.zeros((P, ncols), np.int64)
        rel = np.full((P, ncols), -1.0, np.float32)
        loc = np.zeros((P, ncols), np.int64)
        pos = 0
        for tt in range(n_tiles):
            for ss in range(nseg):
                n = cnts[r, tt, ss]
                if n:
                    j = np.arange(n)
                    col = colbase[tt, ss] + (j >> 7)
                    row = j & 127
                    srcl[row, col] = s[pos:pos + n] - ss * SEGR
                    rel[row, col] = (d[pos:pos + n] - tt * P).astype(np.float32)
                    loc[row, col] = d[pos:pos + n]
                    pos += n
        # F idx stream: per (g,s) instruction, columns in group-staging order
        fparts = []
        aparts = []
        for g_ in range(ngrp):
            gcols = []
            for s_ in range(nseg):
                c0, ncol = finst[g_ * nseg + s_]
                if ncol:
                    cc = np.arange(c0, c0 + ncol)
                    gcols.append(cc)
                    fparts.append(_wrap_idx(
                        srcl[:, cc].T.reshape(-1).astype(np.int16)))
            if need_ald and gcols:
                cc = np.concatenate(gcols)
                aparts.append(_wrap_idx(
                    loc[:, cc].T.reshape(-1).astype(np.int16)))
        ent = {"rel": rel,
               "fidx": np.concatenate(fparts, axis=1)}
        if need_ald:
            ent["aidx"] = np.concatenate(aparts, axis=1)
        cores.append(ent)
    plan = {"nseg": nseg, "ngrp": ngrp, "ncols": ncols, "finst": finst,
            "tile_cols": tile_cols}
    return plan, cores


def _build_and_run(inp):
    import concourse.bass as bass
    import concourse.mybir as mybir
    import concourse.tile as tile
    from concourse import bacc, library_config
    from concourse.masks import make_identity
    from concourse.bass_utils import run_bass_kernel_spmd

    f32, i16 = mybir.dt.float32, mybir.dt.int16
    g = lambda k: np.asarray(inp[k], np.float32)
    gi = lambda k: np.asarray(inp[k], np.int64)

    plw, ew = _prep_edges(gi("writes_src"), gi("writes_dst"), PSH, PT, NA, True)
    plc, ec = _prep_edges(gi("cites_src"), gi("cites_dst"), PSH, PT, NP_, False)
    pla, ea = _prep_edges(gi("auth_src"), gi("auth_dst"), ASH, AT, NP_, True)

    cntc = np.bincount(gi("cites_dst"), minlength=NP_).astype(np.float32)
    rcp = 1.0 / np.maximum(cntc, 1.0)
    rcp_T = np.ones((P, PT * NCORES), np.float32)
    for r in range(NCORES):
        blk = np.pad(rcp[r * PSH:(r + 1) * PSH], (0, PT * P - PSH),
                     constant_values=1.0)
        rcp_T[:, r * PT:(r + 1) * PT] = blk.reshape(PT, P).T

    iota_np = np.tile(np.arange(P, dtype=np.float32), (P, 1))
    wdict = {"iota": iota_np,
             "w_emb_a": g("emb_author_W"), "w_emb_p": g("emb_paper_W"),
             "w_out_a": g("out_author_W"), "w_out_p": g("out_paper_W")}
    for l in range(L):
        uw = (g("gat_writes_Wdst")[l].reshape(H, HEADS, CH)
              * g("gat_writes_adst")[l][None]).sum(-1)
        ua = (g("gat_auth_Wdst")[l].reshape(H, HEADS, CH)
              * g("gat_auth_adst")[l][None]).sum(-1)
        wdict[f"wcat_p{l}"] = np.concatenate(
            [g("gat_auth_Wsrc")[l], uw, g("sage_cites_Wl")[l]], axis=1)
        wdict[f"wcat_a{l}"] = np.concatenate([g("gat_writes_Wsrc")[l], ua],
                                             axis=1)
        wdict[f"wr{l}"] = g("sage_cites_Wr")[l]
        wdict[f"asw{l}"] = np.tile(g("gat_writes_asrc")[l].reshape(1, H), (P, 1))
        wdict[f"asa{l}"] = np.tile(g("gat_auth_asrc")[l].reshape(1, H), (P, 1))

    def rep(v):
        return np.tile(np.asarray(v, np.float32).reshape(1, H), (P, 1))
    nz = lambda v: not np.all(np.asarray(v) == 0.0)
    none1 = lambda v: not np.all(np.asarray(v) == 1.0)
    emb_a_b, emb_p_b = nz(inp["emb_author_b"]), nz(inp["emb_paper_b"])
    out_a_b, out_p_b = nz(inp["out_author_b"]), nz(inp["out_paper_b"])
    bias_p = [g("gat_writes_b")[l] + g("sage_cites_bl")[l]
              + g("sage_cites_br")[l] for l in range(L)]
    bias_a = [g("gat_auth_b")[l] for l in range(L)]
    use_bias_p = [nz(b) for b in bias_p]
    use_bias_a = [nz(b) for b in bias_a]
    use_ln_g = [[none1(g("ln_paper_g")[l]), none1(g("ln_author_g")[l])]
                for l in range(L)]
    use_ln_b = [[nz(g("ln_paper_b")[l]), nz(g("ln_author_b")[l])]
                for l in range(L)]
    for l in range(L):
        if use_bias_p[l]: wdict[f"bias_p{l}"] = rep(bias_p[l])
        if use_bias_a[l]: wdict[f"bias_a{l}"] = rep(bias_a[l])
        if use_ln_g[l][0]: wdict[f"lng_p{l}"] = rep(g("ln_paper_g")[l])
        if use_ln_g[l][1]: wdict[f"lng_a{l}"] = rep(g("ln_author_g")[l])
        if use_ln_b[l][0]: wdict[f"lnb_p{l}"] = rep(g("ln_paper_b")[l])
        if use_ln_b[l][1]: wdict[f"lnb_a{l}"] = rep(g("ln_author_b")[l])
    if emb_a_b: wdict["emb_a_b"] = rep(inp["emb_author_b"])
    if emb_p_b: wdict["emb_p_b"] = rep(inp["emb_paper_b"])
    if out_a_b: wdict["out_a_b"] = rep(inp["out_author_b"])
    if out_p_b: wdict["out_p_b"] = rep(inp["out_paper_b"])

    xa = g("x_author"); xp = g("x_paper")
    in_maps = []
    for r in range(NCORES):
        m = dict(wdict)
        m["x_a"] = np.pad(xa[r * ASH:(r + 1) * ASH], ((0, AT * P - ASH), (0, 0)))
        m["x_p"] = np.pad(xp[r * PSH:(r + 1) * PSH], ((0, PT * P - PSH), (0, 0)))
        m["rel_w"], m["fidx_w"], m["aidx_w"] = \
            ew[r]["rel"], ew[r]["fidx"], ew[r]["aidx"]
        m["rel_c"], m["fidx_c"] = ec[r]["rel"], ec[r]["fidx"]
        m["rel_a"], m["fidx_a"], m["aidx_a"] = \
            ea[r]["rel"], ea[r]["fidx"], ea[r]["aidx"]
        m["rcp_c"] = np.ascontiguousarray(rcp_T[:, r * PT:(r + 1) * PT])
        in_maps.append(m)

    nc = bacc.Bacc("TRN2", target_bir_lowering=False, debug=False,
                   num_devices=NCORES, num_swdge_queues=4)
    ein = lambda n, s, dt=f32: nc.dram_tensor(n, s, dt, kind="ExternalInput").ap()
    eout = lambda n, s: nc.dram_tensor(n, s, f32, kind="ExternalOutput").ap()

    def npdt(v):
        return {np.dtype(np.int16): i16,
                np.dtype(np.float32): f32}[v.dtype]
    d_in = {k: ein(k, list(v.shape), npdt(v)) for k, v in in_maps[0].items()}
    o_a = eout("o_a", [ASH, H])
    o_p = eout("o_p", [PSH, H])

    fa_in = [nc.dram_tensor(f"fa_in{l}", [ASH, EG], f32).ap() for l in range(L)]
    fpg_in = [nc.dram_tensor(f"fpg_in{l}", [PSH, EG], f32).ap() for l in range(L)]
    fps_in = [nc.dram_tensor(f"fps_in{l}", [PSH, ES], f32).ap() for l in range(L)]
    fa_full = [nc.dram_tensor(f"fa_full{l}", [NA, EG], f32,
                              addr_space="Shared").ap() for l in range(L)]
    fpg_full = [nc.dram_tensor(f"fpg_full{l}", [NP_, EG], f32,
                               addr_space="Shared").ap() for l in range(L)]
    fps_full = [nc.dram_tensor(f"fps_full{l}", [NP_, ES], f32,
                               addr_space="Shared").ap() for l in range(L)]
    aldp = [nc.dram_tensor(f"aldp{l}", [PT * P, EA], f32).ap() for l in range(L)]
    alda = [nc.dram_tensor(f"alda{l}", [AT * P, EA], f32).ap() for l in range(L)]
    h_p_d = nc.dram_tensor("h_p_d", [PT * P, H], f32).ap()
    h_a_d = nc.dram_tensor("h_a_d", [AT * P, H], f32).ap()

    RG = [list(range(NCORES))]
    AF = mybir.ActivationFunctionType
    OP = mybir.AluOpType
    h4 = lambda ap: ap.rearrange("p (h c) -> p h c", h=HEADS)
    qrr = {"q": 0}

    def nextq():
        qrr["q"] = (qrr["q"] + 1) % 4
        return qrr["q"]

    with tile.TileContext(nc) as tc:
        with tc.tile_pool(name="const", bufs=1) as cp, \
             tc.tile_pool(name="meta", bufs=1) as mp, \
             tc.tile_pool(name="work", bufs=3) as wp, \
             tc.tile_pool(name="gat", bufs=8) as gp, \
             tc.tile_pool(name="stage", bufs=2) as sp, \
             tc.tile_pool(name="psA", bufs=3, space="PSUM") as psA, \
             tc.tile_pool(name="psB", bufs=2, space="PSUM") as psB:

            nc.gpsimd.load_library(library_config.mlp)

            def cload(name):
                t = cp.tile(list(in_maps[0][name].shape), f32, tag=name,
                            name=name)
                nc.sync.dma_start(t[:], d_in[name][:])
                return t
            ident = cp.tile([P, P], f32, tag="ident")
            make_identity(nc, ident[:])
            eps_t = cp.tile([P, 1], f32, tag="epsc")
            nc.gpsimd.memset(eps_t[:], LN_EPS)
            iota = cload("iota")
            w_emb_a = cload("w_emb_a")
            w_emb_p0 = cp.tile([P, H], f32, tag="wep0")
            w_emb_p1 = cp.tile([P, H], f32, tag="wep1")
            nc.sync.dma_start(w_emb_p0[:], d_in["w_emb_p"][0:P, :])
            nc.sync.dma_start(w_emb_p1[:], d_in["w_emb_p"][P:2 * P, :])
            w_out_a, w_out_p = cload("w_out_a"), cload("w_out_p")
            WS = {k: cload(k) for k in
                  [f"{n}{l}" for l in range(L)
                   for n in ("wcat_p", "wcat_a", "wr", "asw", "asa")]}
            OPT = {k: cload(k) for k in wdict
                   if k.startswith(("bias_", "lng_", "lnb_", "emb_", "out_"))
                   and k in d_in}
            META = {k: mp.tile(list(in_maps[0][k].shape), f32, tag=k, name=k)
                    for k in ("rel_w", "rel_c", "rel_a", "rcp_c")}
            for k, t in META.items():
                nc.sync.dma_start(t[:], d_in[k][:])

            def transpose_to_sbuf(src_ap, tag):
                tp = psA.tile([P, P], f32, tag="T")
                nc.tensor.transpose(out=tp[:], in_=src_ap, identity=ident[:])
                sb = wp.tile([P, P], f32, tag=tag)
                nc.vector.tensor_copy(sb[:], tp[:])
                return sb

            # ---- embeddings -> h DRAM ----
            for t in range(PT):
                xt = wp.tile([P, IN_P], f32, tag="xt")
                nc.sync.dma_start(xt[:], d_in["x_p"][t * P:(t + 1) * P, :])
                tp0 = psA.tile([P, P], f32, tag="T")
                nc.tensor.transpose(out=tp0[:], in_=xt[:, 0:P], identity=ident[:])
                tp1 = psA.tile([P, P], f32, tag="T")
                nc.tensor.transpose(out=tp1[:], in_=xt[:, P:2 * P],
                                    identity=ident[:])
                xT = wp.tile([P, IN_P], f32, tag="xT")
                nc.vector.tensor_copy(xT[:, 0:P], tp0[:])
                nc.vector.tensor_copy(xT[:, P:2 * P], tp1[:])
                hm = psA.tile([P, H], f32, tag="T", name="hm")
                nc.tensor.matmul(out=hm[:], lhsT=xT[:, 0:P], rhs=w_emb_p0[:],
                                 start=True, stop=False)
                nc.tensor.matmul(out=hm[:], lhsT=xT[:, P:2 * P], rhs=w_emb_p1[:],
                                 start=False, stop=True)
                ht = wp.tile([P, H], f32, tag="ht")
                if emb_p_b:
                    nc.vector.tensor_add(ht[:], hm[:], OPT["emb_p_b"][:])
                    nc.scalar.activation(out=ht[:], in_=ht[:], func=AF.Relu)
                else:
                    nc.scalar.activation(out=ht[:], in_=hm[:], func=AF.Relu)
                nc.sync.dma_start(h_p_d[t * P:(t + 1) * P, :], ht[:])
            for t in range(AT):
                xt = wp.tile([P, IN_A], f32, tag="xt")
                nc.sync.dma_start(xt[:], d_in["x_a"][t * P:(t + 1) * P, :])
                xT = transpose_to_sbuf(xt[:, 0:P], "xTa")
                hm = psA.tile([P, H], f32, tag="T", name="hm")
                nc.tensor.matmul(out=hm[:], lhsT=xT[:], rhs=w_emb_a[:],
                                 start=True, stop=True)
                ht = wp.tile([P, H], f32, tag="ht")
                if emb_a_b:
                    nc.vector.tensor_add(ht[:], hm[:], OPT["emb_a_b"][:])
                    nc.scalar.activation(out=ht[:], in_=ht[:], func=AF.Relu)
                else:
                    nc.scalar.activation(out=ht[:], in_=hm[:], func=AF.Relu)
                nc.sync.dma_start(h_a_d[t * P:(t + 1) * P, :], ht[:])

            def f_pass(l, n_tiles, n_rows, h_dram, wcat, wide, asr, f_gat_dram,
                       ald_dram, f_sage_dram):
                for t in range(n_tiles):
                    rows = min(P, n_rows - t * P)
                    htl = wp.tile([P, H], f32, tag="htl")
                    nc.sync.dma_start(htl[:], h_dram[t * P:(t + 1) * P, :])
                    hT = transpose_to_sbuf(htl[:], "hT")
                    fg = psB.tile([P, 260], f32, tag="F", name="fg")
                    nc.tensor.matmul(out=fg[:, 0:wide], lhsT=hT[:], rhs=wcat[:],
                                     start=True, stop=True)
                    als_m = wp.tile([P, H], f32, tag="alsm")
                    nc.vector.tensor_tensor(out=h4(als_m[:]), in0=h4(fg[:, 0:H]),
                                            in1=h4(asr[:]), op=OP.mult)
                    stage = wp.tile([P, AS0 + 4], f32, tag="fstage")
                    nc.scalar.activation(out=stage[:, 0:AS0], in_=fg[:, 0:AS0],
                                         func=AF.Identity)
                    nc.vector.reduce_sum(
                        out=stage[:, AS0:AS0 + 4].unsqueeze(2),
                        in_=h4(als_m[:]), axis=mybir.AxisListType.X)
                    nc.sync.dma_start(f_gat_dram[t * P:t * P + rows, 0:AS0 + 4],
                                      stage[:rows, :])
                    alds = wp.tile([P, 4], f32, tag="alds")
                    nc.vector.tensor_copy(alds[:], fg[:, H:H + 4])
                    nc.sync.dma_start(ald_dram[t * P:(t + 1) * P, 0:4],
                                      alds[:, :])
                    if f_sage_dram is not None:
                        st2 = wp.tile([P, ES], f32, tag="fstage2")
                        nc.scalar.activation(out=st2[:], in_=fg[:, AS0:AS0 + ES],
                                             func=AF.Identity)
                        nc.sync.dma_start(f_sage_dram[t * P:t * P + rows, :],
                                          st2[:rows, :])

            def edge_gathers(plan, fidx_d, f_full, n_rows, elem, stg_tag,
                             aidx_d=None, ald_d=None):
                """Per group: batched dma_gathers. Returns per group
                (colmap: global col -> (stage_tile, pos), ald_tile)."""
                out = []
                fpos = 0
                apos = 0
                nseg = plan["nseg"]
                for g_ in range(plan["ngrp"]):
                    colmap = {}
                    tot = 0
                    stgs = []
                    for s_ in range(nseg):
                        c0, ncol = plan["finst"][g_ * nseg + s_]
                        if ncol == 0:
                            continue
                        stg = sp.tile([P, ncol, elem], f32, tag=stg_tag,
                                      name=stg_tag,
                                      bufs=plan["nseg"] + 2)
                        lo = s_ * SEGR
                        hi = min(lo + SEGR, n_rows)
                        # dma_gather caps at 1024 indices per instruction
                        for p0 in range(0, ncol, 8):
                            pc = min(8, ncol - p0)
                            ni = pc * P
                            it = gp.tile([P, ni // 16], i16,
                                         tag=stg_tag + "ix", name="ixt")
                            nc.sync.dma_start(
                                it[:], fidx_d[:, fpos:fpos + ni // 16])
                            fpos += ni // 16
                            nc.gpsimd.dma_gather(
                                stg[:, p0:p0 + pc, :], f_full[lo:hi, :], it[:],
                                ni, ni, elem, queue_num=nextq())
                        for i in range(ncol):
                            colmap[c0 + i] = (stg, i)
                        stgs.append((stg, ncol))
                        tot += ncol
                    aldt = None
                    if aidx_d is not None and tot:
                        aldt = sp.tile([P, tot, EA], f32, tag=stg_tag + "al",
                                       name="aldt")
                        for p0 in range(0, tot, 8):
                            pc = min(8, tot - p0)
                            ni = pc * P
                            it2 = gp.tile([P, ni // 16], i16,
                                          tag=stg_tag + "ax", name="ixa")
                            nc.sync.dma_start(
                                it2[:], aidx_d[:, apos:apos + ni // 16])
                            apos += ni // 16
                            nc.gpsimd.dma_gather(
                                aldt[:, p0:p0 + pc, :], ald_d[:, :], it2[:],
                                ni, ni, EA, queue_num=nextq())
                    # ald position = sequential over the group's stage order
                    out.append((colmap, aldt))
                return out

            def gat_tile(plan, t, colmap, aldt, aldpos, relm, acc):
                cols = plan["tile_cols"][t]
                n = len(cols)
                for k, c in enumerate(cols):
                    stg, ci = colmap[c]
                    ai = aldpos[c]
                    S = gp.tile([P, P], f32, tag="S")
                    nc.vector.tensor_scalar(out=S[:], in0=iota[:],
                                            scalar1=relm[:, c:c + 1],
                                            scalar2=None, op0=OP.is_equal)
                    e4 = gp.tile([P, 4], f32, tag="e4")
                    nc.vector.tensor_add(e4[:], stg[:, ci, AS0:AS0 + 4],
                                         aldt[:, ai, 0:4])
                    e4b = gp.tile([P, 4], f32, tag="e4b")
                    nc.vector.tensor_scalar(out=e4b[:], in0=e4[:], scalar1=0.2,
                                            scalar2=None, op0=OP.mult)
                    nc.vector.tensor_tensor(out=e4b[:], in0=e4[:], in1=e4b[:],
                                            op=OP.max)
                    msgx = gp.tile([P, H + 4], f32, tag="msg")
                    nc.scalar.activation(out=msgx[:, H:H + 4], in_=e4b[:],
                                         func=AF.Exp)
                    nc.vector.tensor_tensor(
                        out=h4(msgx[:, 0:H]), in0=h4(stg[:, ci, 0:H]),
                        in1=msgx[:, H:H + 4].unsqueeze(2).broadcast_to(
                            [P, HEADS, CH]), op=OP.mult)
                    nc.tensor.matmul(out=acc[:], lhsT=S[:], rhs=msgx[:],
                                     start=(k == 0), stop=(k == n - 1))

            def sage_tile(plan, t, colmap, relm, agg):
                cols = plan["tile_cols"][t]
                n = len(cols)
                for k, c in enumerate(cols):
                    stg, ci = colmap[c]
                    S = gp.tile([P, P], f32, tag="S")
                    nc.vector.tensor_scalar(out=S[:], in0=iota[:],
                                            scalar1=relm[:, c:c + 1],
                                            scalar2=None, op0=OP.is_equal)
                    nc.tensor.matmul(out=agg[:], lhsT=S[:], rhs=stg[:, ci, 0:ES],
                                     start=(k == 0), stop=(k == n - 1))

            def ald_positions(plan):
                """global col -> position within the group's ald staging."""
                maps = []
                nseg = plan["nseg"]
                for g_ in range(plan["ngrp"]):
                    m = {}
                    pos = 0
                    for s_ in range(nseg):
                        c0, ncol = plan["finst"][g_ * nseg + s_]
                        for i in range(ncol):
                            m[c0 + i] = pos + i
                        pos += ncol
                    maps.append(m)
                return maps
            apw, apa = ald_positions(plw), ald_positions(pla)

            def layer_norm_relu_resid(comb, h_dram, t, lng, lnb, htl):
                mus = wp.tile([P, 1], f32, tag="mus")
                nc.vector.reduce_sum(out=mus[:].unsqueeze(2),
                                     in_=comb[:].unsqueeze(1),
                                     axis=mybir.AxisListType.X)
                mu = wp.tile([P, 1], f32, tag="mu")
                nc.vector.tensor_scalar(out=mu[:], in0=mus[:], scalar1=1.0 / H,
                                        scalar2=None, op0=OP.mult)
                nc.vector.tensor_scalar(out=comb[:], in0=comb[:],
                                        scalar1=mu[:, 0:1], scalar2=None,
                                        op0=OP.subtract)
                sqj = wp.tile([P, H], f32, tag="sqj")
                vs = wp.tile([P, 1], f32, tag="vs")
                nc.scalar.activation(out=sqj[:], in_=comb[:], func=AF.Square,
                                     accum_out=vs[:])
                std = wp.tile([P, 1], f32, tag="std")
                nc.scalar.activation(out=std[:], in_=vs[:], func=AF.Sqrt,
                                     scale=1.0 / H, bias=eps_t[:, 0:1])
                rstd = wp.tile([P, 1], f32, tag="rstd")
                nc.vector.reciprocal(rstd[:], std[:])
                nc.vector.tensor_scalar(out=comb[:], in0=comb[:],
                                        scalar1=rstd[:, 0:1], scalar2=None,
                                        op0=OP.mult)
                if lng is not None:
                    nc.vector.tensor_tensor(out=comb[:], in0=comb[:], in1=lng[:],
                                            op=OP.mult)
                if lnb is not None:
                    nc.vector.tensor_add(comb[:], comb[:], lnb[:])
                r = wp.tile([P, H], f32, tag="lnr")
                nc.vector.tensor_scalar(out=r[:], in0=comb[:], scalar1=0.0,
                                        scalar2=None, op0=OP.max)
                hnew = wp.tile([P, H], f32, tag="hnew")
                nc.vector.tensor_add(hnew[:], r[:], htl[:])
                nc.sync.dma_start(h_dram[t * P:(t + 1) * P, :], hnew[:])

            # ---- layers ----
            for l in range(L):
                f_pass(l, PT, PSH, h_p_d, WS[f"wcat_p{l}"], 260, WS[f"asa{l}"],
                       fpg_in[l], aldp[l], fps_in[l])
                f_pass(l, AT, ASH, h_a_d, WS[f"wcat_a{l}"], AS0, WS[f"asw{l}"],
                       fa_in[l], alda[l], None)
                nc.gpsimd.collective_compute(
                    "AllGather", OP.bypass, replica_groups=RG,
                    ins=[fps_in[l][:]], outs=[fps_full[l][:]])
                nc.gpsimd.collective_compute(
                    "AllGather", OP.bypass, replica_groups=RG,
                    ins=[fa_in[l][:]], outs=[fa_full[l][:]])
                nc.gpsimd.collective_compute(
                    "AllGather", OP.bypass, replica_groups=RG,
                    ins=[fpg_in[l][:]], outs=[fpg_full[l][:]])

                cout = edge_gathers(plc, d_in["fidx_c"], fps_full[l], NP_, ES,
                                    "Gc")
                wout = edge_gathers(plw, d_in["fidx_w"], fa_full[l], NA, EG,
                                    "Gw", d_in["aidx_w"], aldp[l])
                for g_ in range(plc["ngrp"]):
                    cmapc, _ = cout[g_]
                    cmapw, aldw = wout[g_]
                    for t in range(g_ * TG, min((g_ + 1) * TG, PT)):
                        comb = wp.tile([P, H], f32, tag="comb")
                        if len(plc["tile_cols"][t]):
                            agg = psB.tile([P, ES], f32, tag="ACC", name="agg",
                                           bufs=3)
                            sage_tile(plc, t, cmapc, META["rel_c"], agg)
                            nc.vector.tensor_scalar(
                                out=comb[:], in0=agg[:],
                                scalar1=META["rcp_c"][:, t:t + 1],
                                scalar2=None, op0=OP.mult)
                        else:
                            nc.gpsimd.memset(comb[:], 0.0)
                        if len(plw["tile_cols"][t]):
                            acc = psB.tile([P, H + 4], f32, tag="ACC",
                                           name="acc", bufs=3)
                            gat_tile(plw, t, cmapw, aldw, apw[g_],
                                     META["rel_w"], acc)
                            s4 = wp.tile([P, 4], f32, tag="s4")
                            nc.vector.tensor_scalar(out=s4[:],
                                                    in0=acc[:, H:H + 4],
                                                    scalar1=1e-16, scalar2=None,
                                                    op0=OP.add)
                            rec = wp.tile([P, 4], f32, tag="rec")
                            nc.vector.reciprocal(rec[:], s4[:])
                            gn = wp.tile([P, H], f32, tag="gn")
                            nc.vector.tensor_tensor(
                                out=h4(gn[:]), in0=h4(acc[:, 0:H]),
                                in1=rec[:].unsqueeze(2).broadcast_to(
                                    [P, HEADS, CH]), op=OP.mult)
                            nc.vector.tensor_add(comb[:], comb[:], gn[:])
                        htl = wp.tile([P, H], f32, tag="htl2")
                        nc.sync.dma_start(htl[:], h_p_d[t * P:(t + 1) * P, :])
                        hT2 = transpose_to_sbuf(htl[:], "hT2")
                        wrp = psA.tile([P, H], f32, tag="T", name="wrp")
                        nc.tensor.matmul(out=wrp[:], lhsT=hT2[:],
                                         rhs=WS[f"wr{l}"][:],
                                         start=True, stop=True)
                        nc.vector.tensor_add(comb[:], comb[:], wrp[:])
                        if use_bias_p[l]:
                            nc.vector.tensor_add(comb[:], comb[:],
                                                 OPT[f"bias_p{l}"][:])
                        layer_norm_relu_resid(
                            comb, h_p_d, t,
                            OPT[f"lng_p{l}"] if use_ln_g[l][0] else None,
                            OPT[f"lnb_p{l}"] if use_ln_b[l][0] else None, htl)

                aout = edge_gathers(pla, d_in["fidx_a"], fpg_full[l], NP_, EG,
                                    "Ga", d_in["aidx_a"], alda[l])
                for g_ in range(pla["ngrp"]):
                    cmapa, aldat = aout[g_]
                    for t in range(g_ * TG, min((g_ + 1) * TG, AT)):
                        comb = wp.tile([P, H], f32, tag="comb")
                        if len(pla["tile_cols"][t]):
                            acc = psB.tile([P, H + 4], f32, tag="ACC",
                                           name="acc", bufs=3)
                            gat_tile(pla, t, cmapa, aldat, apa[g_],
                                     META["rel_a"], acc)
                            s4 = wp.tile([P, 4], f32, tag="s4")
                            nc.vector.tensor_scalar(out=s4[:],
                                                    in0=acc[:, H:H + 4],
                                                    scalar1=1e-16, scalar2=None,
                                                    op0=OP.add)
                            rec = wp.tile([P, 4], f32, tag="rec")
                            nc.vector.reciprocal(rec[:], s4[:])
                            nc.vector.tensor_tensor(
                                out=h4(comb[:]), in0=h4(acc[:, 0:H]),
                                in1=rec[:].unsqueeze(2).broadcast_to(
                                    [P, HEADS, CH]), op=OP.mult)
                        else:
                            nc.gpsimd.memset(comb[:], 0.0)
                        if use_bias_a[l]:
                            nc.vector.tensor_add(comb[:], comb[:],
                                                 OPT[f"bias_a{l}"][:])
                        htl = wp.tile([P, H], f32, tag="htl2")
                        nc.sync.dma_start(htl[:], h_a_d[t * P:(t + 1) * P, :])
                        layer_norm_relu_resid(
                            comb, h_a_d, t,
                            OPT[f"lng_a{l}"] if use_ln_g[l][1] else None,
                            OPT[f"lnb_a{l}"] if use_ln_b[l][1] else None, htl)

            # ---- output projections ----
            for (n_tiles, n_rows, h_dram, w_o, ob_key, use_ob, o_ext) in (
                    (PT, PSH, h_p_d, w_out_p, "out_p_b", out_p_b, o_p),
                    (AT, ASH, h_a_d, w_out_a, "out_a_b", out_a_b, o_a)):
                for t in range(n_tiles):
                    rows = min(P, n_rows - t * P)
                    htl = wp.tile([P, H], f32, tag="htl")
                    nc.sync.dma_start(htl[:], h_dram[t * P:(t + 1) * P, :])
                    hT = transpose_to_sbuf(htl[:], "hTo")
                    om = psA.tile([P, H], f32, tag="T", name="om")
                    nc.tensor.matmul(out=om[:], lhsT=hT[:], rhs=w_o[:],
                                     start=True, stop=True)
                    ost = wp.tile([P, H], f32, tag="ost")
                    if use_ob:
                        nc.vector.tensor_add(ost[:], om[:], OPT[ob_key][:])
                    else:
                        nc.scalar.activation(out=ost[:], in_=om[:],
                                             func=AF.Identity)
                    nc.sync.dma_start(o_ext[t * P:t * P + rows, :],
                                      ost[:rows, :])

    nc.compile()
    res = run_bass_kernel_spmd(nc, in_maps, list(range(NCORES)))
    out_author = np.concatenate([res.results[r]["o_a"] for r in range(NCORES)],
                                0)
    out_paper = np.concatenate([res.results[r]["o_p"] for r in range(NCORES)],
                               0)
    return out_author, out_paper


def kernel(**inputs):
    return _build_and_run(inputs)
